# revision 1
# baseline (speedup 1.0000x reference)
"""BitLinear (ternary weight / int8-activation quantized matmul) Trainium2 kernel.

Reference semantics (for x:(B,S,D), weight:(O,D)):
    alpha = max(mean(|W|), 1e-8)                     # per-tensor scalar
    w_q   = clip(round(W/alpha), -1, 1)              # ternary
    beta  = max(max|x| / 127, 1e-8)                  # per token
    x_q   = clip(round(x/beta), -127, 127)           # int8 range
    y     = (x_q @ w_q.T) * alpha * beta

Sharding: data-parallel over the 16384 tokens across 8 NeuronCores
(2048 tokens/core); full weight replicated per core (no collectives).
The quantized GEMM runs in bf16 which is EXACT here: x_q in [-127,127]
and w_q in {-1,0,1} are exactly representable in bf16 and all partial
sums stay far below 2^24, so fp32 PSUM accumulation is exact.

Rounding uses the fp32 magic-number trick ((v + 1.5*2^23) - 1.5*2^23)
which implements round-half-to-even, matching jnp.round bit-for-bit.
"""

import numpy as np

import bass_rust
import concourse.bass as bass
import concourse.mybir as mybir
import concourse.tile as tile
from concourse.bass_utils import run_bass_kernel_spmd
from concourse.masks import make_identity

N_CORES = 8
P = 128
MAGIC = 12582912.0  # 1.5 * 2**23 : fp32 RNE round-to-integer magic constant
EPS = 1e-8

# Full-problem shapes (hardcoded per the grading contract)
FULL_B, FULL_S, FULL_D = 4, 4096, 2048
D_IN = 2048
D_OUT = 2048
TOK_PER_CORE = FULL_B * FULL_S // N_CORES  # 2048


def _split_excess_waits(nc, max_waits=1):
    """This container's walrus accepts at most `max_waits` sync waits per
    instruction; move excess waits onto preceding same-engine nops."""
    n = 0
    for f in nc.m.functions:
        for bb in f.blocks:
            insts = list(bb.instructions)
            out = []
            changed = False
            for inst in insts:
                si = inst.sync_info
                if si is not None and len(si.on_wait) > max_waits:
                    waits = list(si.on_wait)
                    extra, keep = waits[:-max_waits], waits[-max_waits:]
                    for i in range(0, len(extra), max_waits):
                        chunk = extra[i : i + max_waits]
                        n += 1
                        nop = mybir.InstNoOp(name=f"waitsplit-{n}")
                        nop.engine = inst.engine
                        nop.sync_info = bass_rust.SyncInfo(
                            on_wait=chunk, on_update=[]
                        )
                        out.append(nop)
                    inst.sync_info = bass_rust.SyncInfo(
                        on_wait=keep, on_update=list(si.on_update)
                    )
                    changed = True
                out.append(inst)
            if changed:
                bb.instructions = out


def emit_bitlinear(tc, y_ap, x_ap, wt_ap, d_in, d_out, n_tok):
    """Emit the per-core kernel body.

    x_ap:  [n_tok, d_in]  f32 token rows for this core
    wt_ap: [d_in, d_out]  f32 transposed weight (wt[i,o] = W[o,i])
    y_ap:  [n_tok, d_out] f32 output
    """
    from contextlib import ExitStack

    nc = tc.nc
    f32 = mybir.dt.float32
    bf16 = mybir.dt.bfloat16
    NK = d_in // P
    NO = d_out // 512
    NX = n_tok // P
    inv_n = 1.0 / float(d_in * d_out)  # power of two for our shapes => exact
    NRES = min(4, NK)          # W tiles kept resident from pass 1
    NHEAD = min(6, NX)         # x tiles pre-staged while pass 1 streams
    # k consumption order: resident tiles' W_qT is ready first
    korder = list(range(NK - NRES, NK)) + list(range(NK - NRES))

    with ExitStack() as ctx:
        const = ctx.enter_context(tc.tile_pool(name="const", bufs=1))
        wres = ctx.enter_context(tc.tile_pool(name="wres", bufs=NRES))
        wf32 = ctx.enter_context(tc.tile_pool(name="wf32", bufs=3))
        small = ctx.enter_context(tc.tile_pool(name="small", bufs=10))
        qtmp = ctx.enter_context(tc.tile_pool(name="qtmp", bufs=2))
        trashp = ctx.enter_context(tc.tile_pool(name="trashp", bufs=1))
        wqtp = ctx.enter_context(tc.tile_pool(name="wqtp", bufs=1))
        xf32 = ctx.enter_context(tc.tile_pool(name="xf32", bufs=2))
        xqp = ctx.enter_context(tc.tile_pool(name="xqp", bufs=2))
        xqtp = ctx.enter_context(tc.tile_pool(name="xqtp", bufs=4))
        yout = ctx.enter_context(tc.tile_pool(name="yout", bufs=2))
        pyp = ctx.enter_context(tc.tile_pool(name="pyp", bufs=5, space="PSUM"))
        ptp = ctx.enter_context(tc.tile_pool(name="ptp", bufs=2, space="PSUM"))
        pap = ctx.enter_context(tc.tile_pool(name="pap", bufs=1, space="PSUM"))

        ident = const.tile([P, P], bf16)
        make_identity(nc, ident)
        ones_k = const.tile([P, 1], f32)
        nc.vector.memset(ones_k, 1.0)
        ones_m = const.tile([1, P], f32)
        nc.vector.memset(ones_m, 1.0)

        # ---- Phase 1: alpha = max(mean|W|, EPS).  |W| row-sums on ScalarE
        # (activation Abs + accum_out) at the pass-1 DMA stream cadence.
        # The last NRES tiles stay resident so their quantization can start
        # the moment alpha is known, and the pass-2 re-reads are blocked
        # behind alpha by allocating them from the resident pool's slots.
        partials = const.tile([P, NK], f32)
        wj_res = {}
        for j in range(NK):
            if j >= NK - NRES:
                wj = wres.tile([P, d_out], f32, tag="wr_res", name=f"wres{j}")
                wj_res[j] = wj
            else:
                wj = wf32.tile([P, d_out], f32, tag="wj")
            nc.sync.dma_start(out=wj, in_=wt_ap[j * P : (j + 1) * P, :])
            trash = trashp.tile([P, d_out], bf16, tag="trash")
            nc.scalar.activation(
                out=trash,
                in_=wj,
                func=mybir.ActivationFunctionType.Abs,
                accum_out=partials[:, j : j + 1],
            )

        # ---- head-staged x tiles: quantize + transpose while pass 1 runs.
        # PSUM->SBUF copies for these go on the DVE (ScalarE is busy with
        # the Abs stream).
        def x_quant(i, copies_on):
            xi = xf32.tile([P, d_in], f32, tag="xi", name=f"xi{i}")
            nc.sync.dma_start(out=xi, in_=x_ap[i * P : (i + 1) * P, :])
            am = small.tile([P, 1], f32, tag="am", name=f"am{i}")
            nc.vector.tensor_reduce(
                out=am,
                in_=xi,
                axis=mybir.AxisListType.X,
                op=mybir.AluOpType.max,
                apply_absolute_value=True,
            )
            beta = small.tile([P, 1], f32, tag="beta", name=f"beta{i}")
            nc.vector.tensor_scalar(
                beta, am, 1.0 / 127.0, EPS,
                mybir.AluOpType.mult, mybir.AluOpType.max,
            )
            invb = small.tile([P, 1], f32, tag="invb", name=f"invb{i}")
            nc.vector.reciprocal(out=invb, in_=beta)
            q = qtmp.tile([P, d_in], f32, tag="q32", name=f"xq32{i}")
            nc.vector.tensor_scalar(
                q, xi, invb, MAGIC,
                mybir.AluOpType.mult, mybir.AluOpType.add,
            )
            xq = xqp.tile([P, d_in], bf16, tag="xq", name=f"xq{i}")
            nc.vector.tensor_scalar(
                xq, q, MAGIC, None, mybir.AluOpType.subtract,
            )
            xqt = xqtp.tile([P, NK, P], bf16, tag="xqt", name=f"xqt{i}")
            # 4 transposed k-blocks per PSUM tile (1 bank), evacuated with
            # one wide copy: fewer cross-engine round-trips per x tile.
            GRP = 4
            for g in range((NK + GRP - 1) // GRP):
                n_in_g = min(GRP, NK - g * GRP)
                pt = ptp.tile([P, GRP * P], bf16, tag="pt", name=f"pt{i}_{g}")
                for jj in range(n_in_g):
                    j = g * GRP + jj
                    nc.tensor.transpose(
                        pt[:, jj * P : (jj + 1) * P],
                        xq[:, j * P : (j + 1) * P],
                        ident,
                    )
                dst = xqt[:, g * GRP : g * GRP + n_in_g, :]
                srcv = pt[:, : n_in_g * P]
                if copies_on == "vector" or g % 2 == 0:
                    nc.vector.tensor_copy(dst, srcv)
                else:
                    nc.scalar.copy(out=dst, in_=srcv)
            return beta, xqt

        head = {}
        for i in range(NHEAD):
            head[i] = x_quant(i, "vector")

        # ---- alpha finalize ----
        total = const.tile([P, 1], f32)
        nc.vector.tensor_reduce(
            out=total,
            in_=partials,
            axis=mybir.AxisListType.X,
            op=mybir.AluOpType.add,
        )
        pa_sum = pap.tile([1, 1], f32, tag="pa")
        nc.tensor.matmul(pa_sum, lhsT=total, rhs=ones_k, start=True, stop=True)
        scal = const.tile([1, 2], f32)
        nc.vector.tensor_scalar(
            scal[:, 0:1], pa_sum, inv_n, EPS,
            mybir.AluOpType.mult, mybir.AluOpType.max,
        )
        nc.vector.reciprocal(out=scal[:, 1:2], in_=scal[:, 0:1])
        pa_bc = pap.tile([P, 2], f32, tag="pa")
        nc.tensor.matmul(pa_bc, lhsT=ones_m, rhs=scal, start=True, stop=True)
        ab = const.tile([P, 2], f32)
        nc.scalar.copy(out=ab, in_=pa_bc)
        alpha_bc = ab[:, 0:1]
        invalpha_bc = ab[:, 1:2]

        # ---- Phase 2: W_qT = clip(round(wT * inv_alpha), -1, 1) in bf16,
        # resident tiles first (ready immediately), then the re-reads.
        def w_quant(j, wj):
            q = qtmp.tile([P, d_out], f32, tag="q32", name=f"wq32{j}")
            nc.vector.tensor_scalar(
                q, wj, invalpha_bc, MAGIC,
                mybir.AluOpType.mult, mybir.AluOpType.add,
            )
            r = qtmp.tile([P, d_out], bf16, tag="wr", name=f"wr{j}")
            nc.vector.tensor_scalar(
                r, q, MAGIC, -1.0,
                mybir.AluOpType.subtract, mybir.AluOpType.max,
            )
            wq_j = wqtp.tile([P, d_out], bf16, tag=f"wqt{j}", name=f"wqt{j}")
            nc.vector.tensor_scalar(
                wq_j, r, 1.0, None, mybir.AluOpType.min,
            )
            return wq_j

        wqt_by_k = {}
        for j in range(NK - NRES, NK):
            wqt_by_k[j] = w_quant(j, wj_res[j])
        for j in range(NK - NRES):
            wj = wf32.tile([P, d_out], f32, tag="wj", name=f"wrr{j}")
            nc.sync.dma_start(out=wj, in_=wt_ap[j * P : (j + 1) * P, :])
            wqt_by_k[j] = w_quant(j, wj)

        # ---- Phase 3: matmuls (+ x quant for the non-staged tiles) ----
        for i in range(NX):
            if i in head:
                beta, xqt = head[i]
            else:
                beta, xqt = x_quant(i, "scalar")
            scale = small.tile([P, 1], f32, tag="scale", name=f"scale{i}")
            nc.scalar.mul(out=scale, in_=beta, mul=alpha_bc)
            ysb = yout.tile([P, d_out], f32, tag="ysb")
            if i < 1:
                # ride the W_qT trickle: consume each k slice across all
                # banks as soon as it is quantized
                pys = [
                    pyp.tile([P, 512], f32, tag="py", name=f"py{i}_{b}")
                    for b in range(NO)
                ]
                for idx, k in enumerate(korder):
                    for b in range(NO):
                        nc.tensor.matmul(
                            pys[b],
                            lhsT=xqt[:, k, :],
                            rhs=wqt_by_k[k][:, b * 512 : (b + 1) * 512],
                            start=(idx == 0),
                            stop=(idx == NK - 1),
                        )
                for b in range(NO):
                    nc.scalar.mul(
                        out=ysb[:, b * 512 : (b + 1) * 512],
                        in_=pys[b],
                        mul=scale,
                    )
            else:
                for b in range(NO):
                    py = pyp.tile([P, 512], f32, tag="py")
                    for idx, k in enumerate(korder):
                        nc.tensor.matmul(
                            py,
                            lhsT=xqt[:, k, :],
                            rhs=wqt_by_k[k][:, b * 512 : (b + 1) * 512],
                            start=(idx == 0),
                            stop=(idx == NK - 1),
                        )
                    nc.scalar.mul(
                        out=ysb[:, b * 512 : (b + 1) * 512],
                        in_=py,
                        mul=scale,
                    )
            nc.sync.dma_start(out=y_ap[i * P : (i + 1) * P, :], in_=ysb)


def build_nc(d_in=D_IN, d_out=D_OUT, n_tok=TOK_PER_CORE, n_cores=N_CORES):
    nc = bass.Bass(
        "TRN2", target_bir_lowering=False, debug=False, num_devices=n_cores
    )
    x = nc.dram_tensor("x", [n_tok, d_in], mybir.dt.float32, kind="ExternalInput")
    wt = nc.dram_tensor("wt", [d_in, d_out], mybir.dt.float32, kind="ExternalInput")
    y = nc.dram_tensor("y", [n_tok, d_out], mybir.dt.float32, kind="ExternalOutput")
    with tile.TileContext(nc) as tc:
        emit_bitlinear(tc, y[:, :], x[:, :], wt[:, :], d_in, d_out, n_tok)
    _split_excess_waits(nc)
    return nc


_NC_CACHE = {}


def _run(x: np.ndarray, weight: np.ndarray, **spmd_kwargs):
    x = np.ascontiguousarray(np.asarray(x, dtype=np.float32))
    weight = np.asarray(weight, dtype=np.float32)
    b, s, d = x.shape
    n_tok_full = b * s
    n_tok = n_tok_full // N_CORES
    wt = np.ascontiguousarray(weight.T)

    key = (d, weight.shape[0], n_tok)
    if key not in _NC_CACHE:
        _NC_CACHE[key] = build_nc(d_in=d, d_out=weight.shape[0], n_tok=n_tok)
    nc = _NC_CACHE[key]

    x2d = x.reshape(n_tok_full, d)
    in_maps = [
        {"x": x2d[c * n_tok : (c + 1) * n_tok], "wt": wt} for c in range(N_CORES)
    ]
    res = run_bass_kernel_spmd(
        nc, in_maps, core_ids=list(range(N_CORES)), **spmd_kwargs
    )
    y = np.concatenate([res.results[c]["y"] for c in range(N_CORES)], axis=0)
    return y.reshape(b, s, weight.shape[0]), res


def kernel(x: np.ndarray, weight: np.ndarray) -> np.ndarray:
    y, _ = _run(x, weight)
    return y

